# revision 52
# baseline (speedup 1.0000x reference)
"""Mixtral sparse MoE block with per-expert LoRA on 8 Trainium2 NeuronCores.

Strategy: tensor-parallel shard of the FFN dim F=14336 across 8 cores
(1792 each).  The shared base W1/W3/W2 matmuls (92% of FLOPs) are computed
exactly once per token; per-expert LoRA corrections are applied densely over
experts (routing weights are zero for unselected experts), which keeps the
kernel free of data-dependent gathers.  Per-core partial outputs [H, T] are
summed with a ReduceScatter over the H dim; the host concatenates shards.

Compute dtype fp16 (full-rate on the PE array), fp32 PSUM accumulation.
Layout is feature-major throughout: features on SBUF partitions, tokens on
the free dim, so the routing weights are applied via broadcast tiles.
"""
import sys

sys.path.insert(0, "/opt/trn_rl_repo")

import numpy as np

# ---------------------------------------------------------------- constants
H = 4096
F = 14336
E = 8
R = 32
T = 1024          # B*S tokens
NCORES = 8
FC = F // NCORES  # 1792 F rows per core
MT = FC // 128    # 14 F tiles per core
KH = H // 128     # 32 H (contraction) slices
HT = H // 128     # 32 H output tiles
NCH = 2           # token chunks
CH = T // NCH     # 512 tokens per chunk
ER = E * R        # 256 stacked lora rows


def _build_program():
    import concourse.bacc as bacc
    import concourse.mybir as mybir
    import concourse.tile as tile

    DT = mybir.dt.float16
    F32 = mybir.dt.float32
    AF = mybir.ActivationFunctionType
    OP = mybir.AluOpType

    nc = bacc.Bacc("TRN2", target_bir_lowering=False, debug=False,
                   num_devices=NCORES)

    # ---- dram parameters (per-core shards prepared on the host, fp16)
    xT_d = nc.declare_dram_parameter("xT", [H, T], DT, isOutput=False)
    w1_d = nc.declare_dram_parameter("w1", [MT, 128, KH, 128], DT, isOutput=False)
    w3_d = nc.declare_dram_parameter("w3", [MT, 128, KH, 128], DT, isOutput=False)
    w2_d = nc.declare_dram_parameter("w2", [HT, 128, MT, 128], DT, isOutput=False)
    b1_d = nc.declare_dram_parameter("b1", [MT, 128, E, 128], DT, isOutput=False)
    b3_d = nc.declare_dram_parameter("b3", [MT, 128, E, 128], DT, isOutput=False)
    a2_d = nc.declare_dram_parameter("a2", [128, MT, E, R], DT, isOutput=False)
    b2_d = nc.declare_dram_parameter("b2", [HT, 128, 2, 128], DT, isOutput=False)
    au_d = nc.declare_dram_parameter("au", [KH, 128, 72], DT, isOutput=False)
    we_d = nc.declare_dram_parameter("we", [128, E, T], DT, isOutput=False)

    out_d = nc.declare_dram_parameter("out_shard", [H // NCORES, T], DT,
                                      isOutput=True)
    lg_d = nc.declare_dram_parameter("logitsT", [E, T], F32, isOutput=True)

    # uneven reduce-scatter groups (in h-tiles): big early groups overlap the
    # output phase; tiny final groups minimise the exposed tail
    RS_SIZES = [8, 8, 8, 4, 2, 1, 1]
    assert sum(RS_SIZES) == HT
    RS_START = [sum(RS_SIZES[:i]) for i in range(len(RS_SIZES))]
    rs_in = [nc.dram_tensor(f"rs_in{g}", [n * 128, T], DT)
             for g, n in enumerate(RS_SIZES)]
    rs_out = [nc.dram_tensor(f"rs_out{g}", [n * 128 // NCORES, T], DT)
              for g, n in enumerate(RS_SIZES)]
    agi = [nc.dram_tensor(f"agi{c}", [ER // NCORES * 2, CH], DT)
           for c in range(NCH)]
    ago = [nc.dram_tensor(f"ago{c}", [2 * ER, CH], DT, addr_space="Shared")
           for c in range(NCH)]

    with tile.TileContext(nc) as tc:
        with (
            tc.tile_pool(name="xp", bufs=1) as xp,            # xT chunk 32KB/p
            tc.tile_pool(name="wconst", bufs=1) as wconst,    # resident weights
            tc.tile_pool(name="wstream", bufs=2) as wstream,  # W1/W3/W2 slabs
            tc.tile_pool(name="upool", bufs=1) as upool,
            tc.tile_pool(name="base", bufs=2) as basep,       # x1bs/x3bs
            tc.tile_pool(name="work", bufs=4) as work,        # expert elemwise
            tc.tile_pool(name="x2cp", bufs=1) as x2cp,
            tc.tile_pool(name="vtp", bufs=1) as vtp,
            tc.tile_pool(name="outp", bufs=3) as outp,
            # PSUM: 8 banks total.  ps_b (2, base/out-phase), ps_l (4,
            # logits/u/lora), ps_v (2, held v2 accumulators).
            tc.tile_pool(name="ps_b", bufs=2, space="PSUM") as ps_b,
            tc.tile_pool(name="ps_l", bufs=4, space="PSUM") as ps_l,
            tc.tile_pool(name="ps_v", bufs=1, space="PSUM") as ps_v,
        ):
            # resident small weights (DMAs emitted after the first xT chunk
            # load so the critical-path x load goes out first)
            a2_s = wconst.tile([128, MT, E, R], DT, tag="a2")
            we_s = wconst.tile([128, E, T], DT, tag="we")
            au_s = wconst.tile([128, KH, 72], DT, tag="au")

            x2c_by_chunk = []
            vt_by_chunk = []
            for c in range(NCH):
                tok = slice(c * CH, (c + 1) * CH)
                # ---- load this chunk of xT: [128, KH, CH]
                xt = xp.tile([128, KH, CH], DT, tag="xt")
                for kq in range(4):
                    ks = slice(8 * kq, 8 * (kq + 1))
                    nc.sync.dma_start(
                        xt[:, ks, :],
                        xT_d.rearrange("(k p) t -> p k t", p=128)[:, ks, tok])
                if c == 0:
                    nc.sync.dma_start(au_s[:],
                                      au_d.rearrange("k p j -> p k j"))
                    nc.sync.dma_start(a2_s[:], a2_d[:])
                    nc.sync.dma_start(we_s[:], we_d[:])

                # ---- u1/u3 lora-A projections (this core computes 64 of the
                # 512 stacked rows, AllGathered below) fused with the router
                # logits (8 extra stationary columns)
                u_ps = ps_l.tile([72, CH], F32, tag="lps")
                for k in range(KH):
                    nc.tensor.matmul(u_ps[:], au_s[:, k, :], xt[:, k, :],
                                     start=(k == 0), stop=(k == KH - 1))
                lg_sb = outp.tile([E, CH], F32, tag="lg_sb", bufs=2)
                nc.scalar.activation(lg_sb[:], u_ps[64:72, :], AF.Copy)
                nc.sync.dma_start(lg_d[:, tok], lg_sb[:])
                u_sb = outp.tile([64, CH], DT, tag="u_sb", bufs=2)
                nc.scalar.activation(u_sb[:], u_ps[:64, :], AF.Copy)
                nc.sync.dma_start(agi[c][:], u_sb[:])
                nc.gpsimd.collective_compute(
                    "AllGather", OP.bypass,
                    replica_groups=[list(range(NCORES))],
                    ins=[agi[c][:]], outs=[ago[c][:]],
                )
                u1 = upool.tile([128, 2, CH], DT, tag="u1")
                u3 = upool.tile([128, 2, CH], DT, tag="u3")
                for i in range(2):
                    nc.sync.dma_start(u1[:, i, :],
                                      ago[c][128 * i:128 * (i + 1), :])
                    nc.sync.dma_start(u3[:, i, :],
                                      ago[c][ER + 128 * i:ER + 128 * (i + 1), :])

                # ---- v2 accumulator psums (held across the Ftile loop)
                v2_ps = [ps_v.tile([128, CH], F32, tag=f"v2_{g}",
                                   name=f"v2_ps_{g}")
                         for g in range(2)]

                x2c = x2cp.tile([128, MT, CH], DT, tag=f"x2c{c}")
                x2c_by_chunk.append(x2c)

                def emit_a2(m, x2we_list):
                    # deferred A2 matmuls: rhs tiles are ready by now, so the
                    # in-order PE queue doesn't stall on the DVE chain
                    for e in range(E):
                        b = e % 4
                        g = e // 4
                        bp = slice(32 * b, 32 * (b + 1))
                        nc.tensor.matmul(v2_ps[g][bp, :], a2_s[:, m, e, :],
                                         x2we_list[e][:], start=(m == 0),
                                         stop=(m == MT - 1),
                                         tile_position=(0, 32 * b))

                pending_a2 = None
                for m in range(MT):
                    # base x1b/x3b for this F tile
                    w1sl = wstream.tile([128, KH, 128], DT, tag="w1s")
                    nc.sync.dma_start(w1sl[:], w1_d[m])
                    x1b_ps = ps_b.tile([128, CH], F32, tag="xb")
                    for k in range(KH):
                        nc.tensor.matmul(x1b_ps[:], w1sl[:, k, :], xt[:, k, :],
                                         start=(k == 0), stop=(k == KH - 1))
                    x1bs = basep.tile([128, CH], DT, tag="x1bs")
                    nc.scalar.activation(x1bs[:], x1b_ps[:], AF.Copy)

                    w3sl = wstream.tile([128, KH, 128], DT, tag="w3s")
                    nc.sync.dma_start(w3sl[:], w3_d[m])
                    x3b_ps = ps_b.tile([128, CH], F32, tag="xb")
                    for k in range(KH):
                        nc.tensor.matmul(x3b_ps[:], w3sl[:, k, :], xt[:, k, :],
                                         start=(k == 0), stop=(k == KH - 1))
                    x3bs = basep.tile([128, CH], DT, tag="x3bs")
                    nc.scalar.activation(x3bs[:], x3b_ps[:], AF.Copy)

                    if pending_a2 is not None:
                        emit_a2(m - 1, pending_a2)

                    # zero-padded-to-K=128 lora matmuls: full-array, so they
                    # stream at the clean per-matmul rate with no LDW stalls
                    b1sl = wstream.tile([128, E, 128], DT, tag="b1sl")
                    nc.sync.dma_start(b1sl[:], b1_d[m])
                    b3sl = wstream.tile([128, E, 128], DT, tag="b3sl")
                    nc.sync.dma_start(b3sl[:], b3_d[m])
                    x2we_list = []
                    for e in range(E):
                        g = e // 4
                        l1_ps = ps_l.tile([128, CH], F32, tag="lps")
                        nc.tensor.matmul(l1_ps[:], b1sl[:, e, :],
                                         u1[:, g, :], start=True, stop=True)
                        l3_ps = ps_l.tile([128, CH], F32, tag="lps")
                        nc.tensor.matmul(l3_ps[:], b3sl[:, e, :],
                                         u3[:, g, :], start=True, stop=True)
                        x1e = work.tile([128, CH], DT, tag="x1e")
                        nc.vector.tensor_tensor(x1e[:], l1_ps[:], x1bs[:],
                                                OP.add)
                        s = work.tile([128, CH], DT, tag="s")
                        nc.scalar.activation(s[:], x1e[:], AF.Silu)
                        x3e = work.tile([128, CH], DT, tag="x3e")
                        nc.vector.tensor_tensor(x3e[:], l3_ps[:], x3bs[:],
                                                OP.add)
                        t_ = work.tile([128, CH], DT, tag="t")
                        nc.vector.tensor_tensor(t_[:], s[:], x3e[:], OP.mult)
                        x2we = work.tile([128, CH], DT, tag="x2we", bufs=14)
                        nc.vector.tensor_tensor(x2we[:], t_[:], we_s[:, e, tok],
                                                OP.mult)
                        if e == 0:
                            nc.gpsimd.tensor_copy(x2c[:, m, :], x2we[:])
                        else:
                            nc.gpsimd.tensor_tensor(x2c[:, m, :], x2c[:, m, :],
                                                    x2we[:], OP.add)
                        x2we_list.append(x2we)
                    pending_a2 = x2we_list
                emit_a2(MT - 1, pending_a2)

                # VT in sbuf fp16 [128, 2, CH]
                vt = vtp.tile([128, 2, CH], DT, tag=f"vt{c}")
                for g in range(2):
                    nc.scalar.activation(vt[:, g, :], v2_ps[g][:], AF.Copy)
                vt_by_chunk.append(vt)

            # ---- out = x2c @ W2-shard + VT @ B2stack  -> rs_in (both chunks
            # share each loaded W2 weight tile; reduce-scatter per H-group
            # overlaps the rest of the output phase)
            for h in range(HT):
                w2sl = wstream.tile([128, MT, 128], DT, tag="w2s")
                nc.sync.dma_start(w2sl[:], w2_d[h])
                b2sl = wstream.tile([128, 2, 128], DT, tag="b2s")
                nc.sync.dma_start(b2sl[:], b2_d[h])
                o_ps = [ps_l.tile([128, CH], F32, tag="lps", name=f"o_ps{c}")
                        for c in range(NCH)]
                for kf in range(MT):
                    for c in range(NCH):
                        nc.tensor.matmul(o_ps[c][:], w2sl[:, kf, :],
                                         x2c_by_chunk[c][:, kf, :],
                                         start=(kf == 0), stop=False)
                for g in range(2):
                    for c in range(NCH):
                        nc.tensor.matmul(o_ps[c][:], b2sl[:, g, :],
                                         vt_by_chunk[c][:, g, :],
                                         start=False, stop=(g == 1))
                grp = max(g for g in range(len(RS_SIZES)) if RS_START[g] <= h)
                hh = h - RS_START[grp]
                for c in range(NCH):
                    o_sb = outp.tile([128, CH], DT, tag="o_sb", bufs=2)
                    nc.scalar.activation(o_sb[:], o_ps[c][:], AF.Copy)
                    nc.sync.dma_start(
                        rs_in[grp][128 * hh:128 * (hh + 1),
                                   c * CH:(c + 1) * CH], o_sb[:])
                if h + 1 == RS_START[grp] + RS_SIZES[grp]:
                    nc.gpsimd.collective_compute(
                        "ReduceScatter", OP.add,
                        replica_groups=[list(range(NCORES))],
                        ins=[rs_in[grp][:]], outs=[rs_out[grp][:]],
                    )
                    nc.sync.dma_start(
                        out_d[RS_START[grp] * 16:
                              (RS_START[grp] + RS_SIZES[grp]) * 16, :],
                        rs_out[grp][:])

    nc.compile()
    return nc


_NC_CACHE = None
LAST_RESULT = None  # BassKernelResults of the most recent kernel() call


def _get_program():
    global _NC_CACHE
    if _NC_CACHE is None:
        _NC_CACHE = _build_program()
    return _NC_CACHE


def _routing(x, gate_w):
    """Exact fp32 replica of the reference routing; returns cw [T, E] fp32."""
    logits = x.astype(np.float32) @ gate_w.astype(np.float32).T      # [T, E]
    m = logits.max(axis=-1, keepdims=True)
    ex = np.exp(logits - m)
    probs = ex / ex.sum(axis=-1, keepdims=True)
    i1 = probs.argmax(axis=-1)
    p1 = probs[np.arange(T), i1]
    masked = probs.copy()
    masked[np.arange(T), i1] = -np.inf
    i2 = masked.argmax(axis=-1)
    p2 = probs[np.arange(T), i2]
    tot = p1 + p2
    cw = np.zeros((T, E), np.float32)
    cw[np.arange(T), i1] = p1 / tot
    cw[np.arange(T), i2] = p2 / tot
    return cw


def kernel(hidden_states, gate_w, W1, W2, W3, A1, B1, A2, B2, A3, B3):
    from concourse.bass_utils import run_bass_kernel_spmd

    x = np.asarray(hidden_states, np.float32).reshape(T, H)
    gate_w = np.asarray(gate_w, np.float32)
    W1 = np.asarray(W1, np.float32)
    W2 = np.asarray(W2, np.float32)
    W3 = np.asarray(W3, np.float32)
    A1 = np.asarray(A1, np.float32)
    B1 = np.asarray(B1, np.float32)
    A2 = np.asarray(A2, np.float32)
    B2 = np.asarray(B2, np.float32)
    A3 = np.asarray(A3, np.float32)
    B3 = np.asarray(B3, np.float32)

    cw = _routing(x, gate_w)

    f16 = np.float16
    c_ = np.ascontiguousarray

    def pad_b(B, fc):
        # zero-padded lora-B slabs: expert e's [R, 128] block sits at
        # partition rows 32*(e%4) so a full K=128 matmul against
        # u[:, e//4, :] contracts exactly that expert's rank-32 rows
        out = np.zeros((MT, 128, E, 128), f16)
        blk = B[:, fc, :].reshape(E, MT, 128, R).transpose(0, 1, 3, 2)
        for e in range(E):
            b = e % 4
            out[:, 32 * b:32 * (b + 1), e, :] = blk[e]
        return out

    # shared (identical on every core) prepped arrays
    xT = c_(x.T.astype(f16))                                     # [H, T]
    u_all = np.concatenate(
        [A1.transpose(2, 0, 1).reshape(H, ER),
         A3.transpose(2, 0, 1).reshape(H, ER)], axis=1)          # [H, 512]
    wep = c_(np.broadcast_to(cw.T[None, :, :], (128, E, T)).astype(f16))
    b2p = c_(B2.transpose(0, 2, 1).reshape(ER, H)
             .reshape(2, 128, HT, 128).transpose(2, 1, 0, 3).astype(f16))

    in_maps = []
    for core in range(NCORES):
        fc = slice(core * FC, (core + 1) * FC)
        w1p = c_(W1[fc].reshape(MT, 128, KH, 128)
                 .transpose(0, 3, 2, 1).astype(f16))
        w3p = c_(W3[fc].reshape(MT, 128, KH, 128)
                 .transpose(0, 3, 2, 1).astype(f16))
        w2p = c_(W2[:, fc].reshape(HT, 128, MT, 128)
                 .transpose(0, 3, 2, 1).astype(f16))
        b1p = pad_b(B1, fc)
        b3p = pad_b(B3, fc)
        a2p = c_(A2[:, :, fc].reshape(E, R, MT, 128)
                 .transpose(3, 2, 0, 1).astype(f16))
        aup = c_(np.concatenate(
            [u_all[:, 64 * core:64 * (core + 1)], gate_w.T], axis=1)
            .reshape(KH, 128, 72).astype(f16))
        in_maps.append({
            "xT": xT, "w1": w1p, "w3": w3p, "w2": w2p,
            "b1": b1p, "b3": b3p, "a2": a2p, "b2": b2p,
            "au": aup, "we": wep,
        })

    nc = _get_program()
    res = run_bass_kernel_spmd(nc, in_maps, list(range(NCORES)))
    global LAST_RESULT
    LAST_RESULT = res

    # reassemble: core c's shard holds one strip per reduce-scatter group;
    # group g covers global H rows [128*start_g, 128*(start_g+n_g))
    RS_SIZES = [8, 8, 8, 4, 2, 1, 1]
    out_T = np.empty((H, T), np.float32)
    for c in range(NCORES):
        sh = res.results[c]["out_shard"].astype(np.float32)   # [512, T]
        start = 0
        for n in RS_SIZES:
            strip = n * 16
            out_T[start * 128 + c * strip:
                  start * 128 + (c + 1) * strip] = \
                sh[start * 16:start * 16 + strip]
            start += n
    final = out_T.T.reshape(1, T, H).astype(np.float32)
    logits = res.results[0]["logitsT"].T.astype(np.float32)    # [T, E]
    return final, logits


# revision 53
# speedup vs baseline: 1.1077x; 1.1077x over previous
"""Mixtral sparse MoE block with per-expert LoRA on 8 Trainium2 NeuronCores.

Strategy: tensor-parallel shard of the FFN dim F=14336 across 8 cores
(1792 each).  The shared base W1/W3/W2 matmuls (92% of FLOPs) are computed
exactly once per token; per-expert LoRA corrections are applied densely over
experts (routing weights are zero for unselected experts), which keeps the
kernel free of data-dependent gathers.  Per-core partial outputs [H, T] are
summed with a ReduceScatter over the H dim; the host concatenates shards.

Compute dtype fp16 (full-rate on the PE array), fp32 PSUM accumulation.
Layout is feature-major throughout: features on SBUF partitions, tokens on
the free dim, so the routing weights are applied via broadcast tiles.
"""
import sys

sys.path.insert(0, "/opt/trn_rl_repo")

import numpy as np

# ---------------------------------------------------------------- constants
H = 4096
F = 14336
E = 8
R = 32
T = 1024          # B*S tokens
NCORES = 8
FC = F // NCORES  # 1792 F rows per core
MT = FC // 128    # 14 F tiles per core
KH = H // 128     # 32 H (contraction) slices
HT = H // 128     # 32 H output tiles
NCH = 2           # token chunks
CH = T // NCH     # 512 tokens per chunk
ER = E * R        # 256 stacked lora rows


def _build_program():
    import concourse.bacc as bacc
    import concourse.mybir as mybir
    import concourse.tile as tile

    DT = mybir.dt.float16
    F32 = mybir.dt.float32
    AF = mybir.ActivationFunctionType
    OP = mybir.AluOpType

    nc = bacc.Bacc("TRN2", target_bir_lowering=False, debug=False,
                   num_devices=NCORES)

    # ---- dram parameters (per-core shards prepared on the host, fp16)
    xT_d = nc.declare_dram_parameter("xT", [H, T], DT, isOutput=False)
    w1_d = nc.declare_dram_parameter("w1", [MT, 128, KH, 128], DT, isOutput=False)
    w3_d = nc.declare_dram_parameter("w3", [MT, 128, KH, 128], DT, isOutput=False)
    w2_d = nc.declare_dram_parameter("w2", [HT, 128, MT, 128], DT, isOutput=False)
    b1_d = nc.declare_dram_parameter("b1", [MT, 128, E, 128], DT, isOutput=False)
    b3_d = nc.declare_dram_parameter("b3", [MT, 128, E, 128], DT, isOutput=False)
    a2_d = nc.declare_dram_parameter("a2", [128, MT, E, R], DT, isOutput=False)
    b2_d = nc.declare_dram_parameter("b2", [HT, 128, 2, 128], DT, isOutput=False)
    au_d = nc.declare_dram_parameter("au", [KH, 128, 72], DT, isOutput=False)
    we_d = nc.declare_dram_parameter("we", [128, E, T], DT, isOutput=False)

    out_d = nc.declare_dram_parameter("out_shard", [H // NCORES, T], DT,
                                      isOutput=True)
    lg_d = nc.declare_dram_parameter("logitsT", [E, T], F32, isOutput=True)

    # uneven reduce-scatter groups (in h-tiles): big early groups overlap the
    # output phase; tiny final groups minimise the exposed tail
    RS_SIZES = [4, 4, 4, 4, 4, 4, 4, 4]
    assert sum(RS_SIZES) == HT
    RS_START = [sum(RS_SIZES[:i]) for i in range(len(RS_SIZES))]
    rs_in = [nc.dram_tensor(f"rs_in{g}", [n * 128, T], DT)
             for g, n in enumerate(RS_SIZES)]
    rs_out = [nc.dram_tensor(f"rs_out{g}", [n * 128 // NCORES, T], DT)
              for g, n in enumerate(RS_SIZES)]
    agi = [nc.dram_tensor(f"agi{c}", [ER // NCORES * 2, CH], DT)
           for c in range(NCH)]
    ago = [nc.dram_tensor(f"ago{c}", [2 * ER, CH], DT, addr_space="Shared")
           for c in range(NCH)]

    with tile.TileContext(nc) as tc:
        with (
            tc.tile_pool(name="xp", bufs=1) as xp,            # xT chunk 32KB/p
            tc.tile_pool(name="wconst", bufs=1) as wconst,    # resident weights
            tc.tile_pool(name="wstream", bufs=2) as wstream,  # W1/W3/W2 slabs
            tc.tile_pool(name="upool", bufs=1) as upool,
            tc.tile_pool(name="base", bufs=2) as basep,       # x1bs/x3bs
            tc.tile_pool(name="work", bufs=4) as work,        # expert elemwise
            tc.tile_pool(name="x2cp", bufs=1) as x2cp,
            tc.tile_pool(name="vtp", bufs=1) as vtp,
            tc.tile_pool(name="outp", bufs=3) as outp,
            # PSUM: 8 banks total.  ps_b (2, base/out-phase), ps_l (4,
            # logits/u/lora), ps_v (2, held v2 accumulators).
            tc.tile_pool(name="ps_b", bufs=2, space="PSUM") as ps_b,
            tc.tile_pool(name="ps_l", bufs=4, space="PSUM") as ps_l,
            tc.tile_pool(name="ps_v", bufs=1, space="PSUM") as ps_v,
        ):
            # resident small weights (DMAs emitted after the first xT chunk
            # load so the critical-path x load goes out first)
            a2_s = wconst.tile([128, MT, E, R], DT, tag="a2")
            we_s = wconst.tile([128, E, T], DT, tag="we")
            au_s = wconst.tile([128, KH, 72], DT, tag="au")

            x2c_by_chunk = []
            vt_by_chunk = []
            for c in range(NCH):
                tok = slice(c * CH, (c + 1) * CH)
                # ---- load this chunk of xT: [128, KH, CH]
                xt = xp.tile([128, KH, CH], DT, tag="xt")
                for kq in range(4):
                    ks = slice(8 * kq, 8 * (kq + 1))
                    nc.sync.dma_start(
                        xt[:, ks, :],
                        xT_d.rearrange("(k p) t -> p k t", p=128)[:, ks, tok])
                if c == 0:
                    nc.sync.dma_start(au_s[:],
                                      au_d.rearrange("k p j -> p k j"))
                    nc.sync.dma_start(a2_s[:], a2_d[:])
                    nc.sync.dma_start(we_s[:], we_d[:])

                # ---- u1/u3 lora-A projections (this core computes 64 of the
                # 512 stacked rows, AllGathered below) fused with the router
                # logits (8 extra stationary columns)
                u_ps = ps_l.tile([72, CH], F32, tag="lps")
                for k in range(KH):
                    nc.tensor.matmul(u_ps[:], au_s[:, k, :], xt[:, k, :],
                                     start=(k == 0), stop=(k == KH - 1))
                lg_sb = outp.tile([E, CH], F32, tag="lg_sb", bufs=2)
                nc.scalar.activation(lg_sb[:], u_ps[64:72, :], AF.Copy)
                nc.sync.dma_start(lg_d[:, tok], lg_sb[:])
                u_sb = outp.tile([64, CH], DT, tag="u_sb", bufs=2)
                nc.scalar.activation(u_sb[:], u_ps[:64, :], AF.Copy)
                nc.sync.dma_start(agi[c][:], u_sb[:])
                nc.gpsimd.collective_compute(
                    "AllGather", OP.bypass,
                    replica_groups=[list(range(NCORES))],
                    ins=[agi[c][:]], outs=[ago[c][:]],
                )
                u1 = upool.tile([128, 2, CH], DT, tag="u1")
                u3 = upool.tile([128, 2, CH], DT, tag="u3")
                for i in range(2):
                    nc.sync.dma_start(u1[:, i, :],
                                      ago[c][128 * i:128 * (i + 1), :])
                    nc.sync.dma_start(u3[:, i, :],
                                      ago[c][ER + 128 * i:ER + 128 * (i + 1), :])

                # ---- v2 accumulator psums (held across the Ftile loop)
                v2_ps = [ps_v.tile([128, CH], F32, tag=f"v2_{g}",
                                   name=f"v2_ps_{g}")
                         for g in range(2)]

                x2c = x2cp.tile([128, MT, CH], DT, tag=f"x2c{c}")
                x2c_by_chunk.append(x2c)

                def emit_a2(m, x2we_list):
                    # deferred A2 matmuls: rhs tiles are ready by now, so the
                    # in-order PE queue doesn't stall on the DVE chain
                    for e in range(E):
                        b = e % 4
                        g = e // 4
                        bp = slice(32 * b, 32 * (b + 1))
                        nc.tensor.matmul(v2_ps[g][bp, :], a2_s[:, m, e, :],
                                         x2we_list[e][:], start=(m == 0),
                                         stop=(m == MT - 1),
                                         tile_position=(0, 32 * b))

                pending_a2 = None
                for m in range(MT):
                    # base x1b/x3b for this F tile
                    w1sl = wstream.tile([128, KH, 128], DT, tag="w1s")
                    nc.sync.dma_start(w1sl[:], w1_d[m])
                    x1b_ps = ps_b.tile([128, CH], F32, tag="xb")
                    for k in range(KH):
                        nc.tensor.matmul(x1b_ps[:], w1sl[:, k, :], xt[:, k, :],
                                         start=(k == 0), stop=(k == KH - 1))
                    x1bs = basep.tile([128, CH], DT, tag="x1bs")
                    nc.scalar.activation(x1bs[:], x1b_ps[:], AF.Copy)

                    w3sl = wstream.tile([128, KH, 128], DT, tag="w3s")
                    nc.sync.dma_start(w3sl[:], w3_d[m])
                    x3b_ps = ps_b.tile([128, CH], F32, tag="xb")
                    for k in range(KH):
                        nc.tensor.matmul(x3b_ps[:], w3sl[:, k, :], xt[:, k, :],
                                         start=(k == 0), stop=(k == KH - 1))
                    x3bs = basep.tile([128, CH], DT, tag="x3bs")
                    nc.scalar.activation(x3bs[:], x3b_ps[:], AF.Copy)

                    if pending_a2 is not None:
                        emit_a2(m - 1, pending_a2)

                    # zero-padded-to-K=128 lora matmuls: full-array, so they
                    # stream at the clean per-matmul rate with no LDW stalls
                    b1sl = wstream.tile([128, E, 128], DT, tag="b1sl")
                    nc.sync.dma_start(b1sl[:], b1_d[m])
                    b3sl = wstream.tile([128, E, 128], DT, tag="b3sl")
                    nc.sync.dma_start(b3sl[:], b3_d[m])
                    x2we_list = []
                    for e in range(E):
                        g = e // 4
                        l1_ps = ps_l.tile([128, CH], F32, tag="lps")
                        nc.tensor.matmul(l1_ps[:], b1sl[:, e, :],
                                         u1[:, g, :], start=True, stop=True)
                        l3_ps = ps_l.tile([128, CH], F32, tag="lps")
                        nc.tensor.matmul(l3_ps[:], b3sl[:, e, :],
                                         u3[:, g, :], start=True, stop=True)
                        x1e = work.tile([128, CH], DT, tag="x1e")
                        nc.vector.tensor_tensor(x1e[:], l1_ps[:], x1bs[:],
                                                OP.add)
                        s = work.tile([128, CH], DT, tag="s")
                        nc.scalar.activation(s[:], x1e[:], AF.Silu)
                        x3e = work.tile([128, CH], DT, tag="x3e")
                        nc.vector.tensor_tensor(x3e[:], l3_ps[:], x3bs[:],
                                                OP.add)
                        t_ = work.tile([128, CH], DT, tag="t")
                        nc.vector.tensor_tensor(t_[:], s[:], x3e[:], OP.mult)
                        x2we = work.tile([128, CH], DT, tag="x2we", bufs=14)
                        nc.vector.tensor_tensor(x2we[:], t_[:], we_s[:, e, tok],
                                                OP.mult)
                        if e == 0:
                            nc.gpsimd.tensor_copy(x2c[:, m, :], x2we[:])
                        else:
                            nc.gpsimd.tensor_tensor(x2c[:, m, :], x2c[:, m, :],
                                                    x2we[:], OP.add)
                        x2we_list.append(x2we)
                    pending_a2 = x2we_list
                emit_a2(MT - 1, pending_a2)

                # VT in sbuf fp16 [128, 2, CH]
                vt = vtp.tile([128, 2, CH], DT, tag=f"vt{c}")
                for g in range(2):
                    nc.scalar.activation(vt[:, g, :], v2_ps[g][:], AF.Copy)
                vt_by_chunk.append(vt)

            # ---- out = x2c @ W2-shard + VT @ B2stack  -> rs_in (both chunks
            # share each loaded W2 weight tile; reduce-scatter per H-group
            # overlaps the rest of the output phase)
            for h in range(HT):
                w2sl = wstream.tile([128, MT, 128], DT, tag="w2s")
                nc.sync.dma_start(w2sl[:], w2_d[h])
                b2sl = wstream.tile([128, 2, 128], DT, tag="b2s")
                nc.sync.dma_start(b2sl[:], b2_d[h])
                o_ps = [ps_l.tile([128, CH], F32, tag="lps", name=f"o_ps{c}")
                        for c in range(NCH)]
                for kf in range(MT):
                    for c in range(NCH):
                        nc.tensor.matmul(o_ps[c][:], w2sl[:, kf, :],
                                         x2c_by_chunk[c][:, kf, :],
                                         start=(kf == 0), stop=False)
                for g in range(2):
                    for c in range(NCH):
                        nc.tensor.matmul(o_ps[c][:], b2sl[:, g, :],
                                         vt_by_chunk[c][:, g, :],
                                         start=False, stop=(g == 1))
                grp = max(g for g in range(len(RS_SIZES)) if RS_START[g] <= h)
                hh = h - RS_START[grp]
                for c in range(NCH):
                    o_sb = outp.tile([128, CH], DT, tag="o_sb", bufs=2)
                    nc.scalar.activation(o_sb[:], o_ps[c][:], AF.Copy)
                    nc.sync.dma_start(
                        rs_in[grp][128 * hh:128 * (hh + 1),
                                   c * CH:(c + 1) * CH], o_sb[:])
                if h + 1 == RS_START[grp] + RS_SIZES[grp]:
                    nc.gpsimd.collective_compute(
                        "ReduceScatter", OP.add,
                        replica_groups=[list(range(NCORES))],
                        ins=[rs_in[grp][:]], outs=[rs_out[grp][:]],
                    )
                    nc.sync.dma_start(
                        out_d[RS_START[grp] * 16:
                              (RS_START[grp] + RS_SIZES[grp]) * 16, :],
                        rs_out[grp][:])

    nc.compile()
    return nc


_NC_CACHE = None
LAST_RESULT = None  # BassKernelResults of the most recent kernel() call


def _get_program():
    global _NC_CACHE
    if _NC_CACHE is None:
        _NC_CACHE = _build_program()
    return _NC_CACHE


def _routing(x, gate_w):
    """Exact fp32 replica of the reference routing; returns cw [T, E] fp32."""
    logits = x.astype(np.float32) @ gate_w.astype(np.float32).T      # [T, E]
    m = logits.max(axis=-1, keepdims=True)
    ex = np.exp(logits - m)
    probs = ex / ex.sum(axis=-1, keepdims=True)
    i1 = probs.argmax(axis=-1)
    p1 = probs[np.arange(T), i1]
    masked = probs.copy()
    masked[np.arange(T), i1] = -np.inf
    i2 = masked.argmax(axis=-1)
    p2 = probs[np.arange(T), i2]
    tot = p1 + p2
    cw = np.zeros((T, E), np.float32)
    cw[np.arange(T), i1] = p1 / tot
    cw[np.arange(T), i2] = p2 / tot
    return cw


def kernel(hidden_states, gate_w, W1, W2, W3, A1, B1, A2, B2, A3, B3):
    from concourse.bass_utils import run_bass_kernel_spmd

    x = np.asarray(hidden_states, np.float32).reshape(T, H)
    gate_w = np.asarray(gate_w, np.float32)
    W1 = np.asarray(W1, np.float32)
    W2 = np.asarray(W2, np.float32)
    W3 = np.asarray(W3, np.float32)
    A1 = np.asarray(A1, np.float32)
    B1 = np.asarray(B1, np.float32)
    A2 = np.asarray(A2, np.float32)
    B2 = np.asarray(B2, np.float32)
    A3 = np.asarray(A3, np.float32)
    B3 = np.asarray(B3, np.float32)

    cw = _routing(x, gate_w)

    f16 = np.float16
    c_ = np.ascontiguousarray

    def pad_b(B, fc):
        # zero-padded lora-B slabs: expert e's [R, 128] block sits at
        # partition rows 32*(e%4) so a full K=128 matmul against
        # u[:, e//4, :] contracts exactly that expert's rank-32 rows
        out = np.zeros((MT, 128, E, 128), f16)
        blk = B[:, fc, :].reshape(E, MT, 128, R).transpose(0, 1, 3, 2)
        for e in range(E):
            b = e % 4
            out[:, 32 * b:32 * (b + 1), e, :] = blk[e]
        return out

    # shared (identical on every core) prepped arrays
    xT = c_(x.T.astype(f16))                                     # [H, T]
    u_all = np.concatenate(
        [A1.transpose(2, 0, 1).reshape(H, ER),
         A3.transpose(2, 0, 1).reshape(H, ER)], axis=1)          # [H, 512]
    wep = c_(np.broadcast_to(cw.T[None, :, :], (128, E, T)).astype(f16))
    b2p = c_(B2.transpose(0, 2, 1).reshape(ER, H)
             .reshape(2, 128, HT, 128).transpose(2, 1, 0, 3).astype(f16))

    in_maps = []
    for core in range(NCORES):
        fc = slice(core * FC, (core + 1) * FC)
        w1p = c_(W1[fc].reshape(MT, 128, KH, 128)
                 .transpose(0, 3, 2, 1).astype(f16))
        w3p = c_(W3[fc].reshape(MT, 128, KH, 128)
                 .transpose(0, 3, 2, 1).astype(f16))
        w2p = c_(W2[:, fc].reshape(HT, 128, MT, 128)
                 .transpose(0, 3, 2, 1).astype(f16))
        b1p = pad_b(B1, fc)
        b3p = pad_b(B3, fc)
        a2p = c_(A2[:, :, fc].reshape(E, R, MT, 128)
                 .transpose(3, 2, 0, 1).astype(f16))
        aup = c_(np.concatenate(
            [u_all[:, 64 * core:64 * (core + 1)], gate_w.T], axis=1)
            .reshape(KH, 128, 72).astype(f16))
        in_maps.append({
            "xT": xT, "w1": w1p, "w3": w3p, "w2": w2p,
            "b1": b1p, "b3": b3p, "a2": a2p, "b2": b2p,
            "au": aup, "we": wep,
        })

    nc = _get_program()
    res = run_bass_kernel_spmd(nc, in_maps, list(range(NCORES)))
    global LAST_RESULT
    LAST_RESULT = res

    # reassemble: core c's shard holds one strip per reduce-scatter group;
    # group g covers global H rows [128*start_g, 128*(start_g+n_g))
    RS_SIZES = [4, 4, 4, 4, 4, 4, 4, 4]
    out_T = np.empty((H, T), np.float32)
    for c in range(NCORES):
        sh = res.results[c]["out_shard"].astype(np.float32)   # [512, T]
        start = 0
        for n in RS_SIZES:
            strip = n * 16
            out_T[start * 128 + c * strip:
                  start * 128 + (c + 1) * strip] = \
                sh[start * 16:start * 16 + strip]
            start += n
    final = out_T.T.reshape(1, T, H).astype(np.float32)
    logits = res.results[0]["logitsT"].T.astype(np.float32)    # [T, E]
    return final, logits
